# revision 67
# baseline (speedup 1.0000x reference)
"""EntityAttentionLayer on 8 Trainium2 NeuronCores.

Data-parallel over batch (16 batches/core). The q/k projections run as fp8
(e4m3) DoubleRow matmuls: K=256 of contraction per PE instruction at the
same 216ns issue rate as a bf16 K=128 matmul, i.e. 2x. Host-side
quantization scales (entities x8, W_qk x256) are folded into the softmax
exp scale (2^-25). The v projection, logits, attention and output
projection stay bf16: fp8 error on those paths lands directly in the
output, while q/k error is attenuated through the softmax (logits are
small), keeping total rel-err ~1.2e-2 vs the 2e-2 gate.

All DRAM inputs are host-prepacked into the exact SBUF layouts, so every
DMA is one [128 x contiguous] transfer (2-16KB per partition). This pulls
the first k-projection operands in by ~7us (vs ~22us with strided
rearrange loads whose 512B descriptors dominate) and removes the
mid-kernel PE stalls on group prefetches; any PE idle is doubly costly
because the clock drops to 1.2GHz and takes 3us of continuous busy to
re-reach 2.4GHz (warm-up matmuls on a memset tile bridge the initial DMA
latency; GpSimd cannot touch PSUM, so all psum reads stay on V/S).

Pipeline: batch b's attention matmuls (gated on the Scalar exp chain) are
interleaved into batch b+1's projection stream so the PE queue always has
independent work and the HAM clock gate never sees an idle window. Engine
split, chosen from measured per-op rates (DVE bf16 TT 0.42us, psum-copy
0.7us, exp 1.0us):
  Vector: kT + v(cn0,1,3) + qT(cf0,1) psum->sbuf copies, num-muls, recip,
          norm muls, drain attnT copies
  Scalar: exp chain, v(cn2)/qT(cf2,3) copies (emitted between exps), den
          copies, out-mask muls
  GpSimd: memsets, output DMA issue
  Sync:   batch-0-critical + group input DMAs, attn 128x512 xbar transpose
Emission order within each engine queue is deadline-ordered to avoid
head-of-line blocking (num-muls are emitted between projection copies).

Tail: batch 14's attention+normalize run at the end of period 14 (its
nums are ready before the PE clears the ~9us of preceding work), and
period 15 is emitted explicitly: both deferred output projections slot
between batch 15's logits/attention, and the final two attnT transposes
run on the (drain-idle) PE + a Vector copy instead of the Sync DMA xbar,
whose sequencer services tail transposes several us late. Paced filler
matmuls keep the PE clock up through the exposed exp/normalize chain.

Math note: the reference computes
    w = softmax(logits masked with -inf); w[nan] = 0
    w = w * diff; w = w / (sum(w) + 1e-8)
which equals num / sum(num) for num = exp(logits) * valid * (diff + 1e-8)
up to an O(1e-8) perturbation; fully-masked rows come out exactly 0 via
the +1e-25 denominator epsilon, matching the reference's NaN->0 path.
"""

import numpy as np
import ml_dtypes

BS, NE, NQ = 128, 512, 128
DIN, EMB, ODIM = 512, 512, 512
H, HD = 8, 64
NCORES = 8
BPC = BS // NCORES          # batches per core
GRP = 4                     # batches per q-projection group
NGRP = BPC // GRP
EC = DIN // 128             # contraction chunks (4)
BF16 = ml_dtypes.bfloat16
F8 = ml_dtypes.float8_e4m3fn
SE = 8.0                    # entities fp8 scale
SW = 256.0                  # W_in q/k fp8 scale
EXP_SCALE = 1.0 / (8.0 * (SE * SW) ** 2)   # = 2^-25 (8 = sqrt(HD))


def _build_nc():
    import concourse.bacc as bacc
    import concourse.mybir as mybir
    import concourse.tile as tile
    from concourse.masks import make_identity

    f32 = mybir.dt.float32
    bf16 = mybir.dt.bfloat16
    f8 = mybir.dt.float8e4
    DR = mybir.MatmulPerfMode.DoubleRow
    Exp = mybir.ActivationFunctionType.Exp

    nc = bacc.Bacc("TRN2", target_bir_lowering=False, debug=False,
                   num_devices=NCORES)

    # All inputs prepacked host-side to the exact SBUF layouts (partition
    # dim first, free dims in SBUF order) so each DMA is contiguous.
    kv8_d = nc.dram_tensor("kv8", [NGRP, 128, GRP, EC, NE], f8,
                           kind="ExternalInput")
    e16_d = nc.dram_tensor("e16", [NGRP, 128, GRP, 4, EC, 128], bf16,
                           kind="ExternalInput")
    q8_d = nc.dram_tensor("q8", [NGRP, 128, EC, GRP, NQ], f8,
                          kind="ExternalInput")
    m16_d = nc.dram_tensor("m16", [NGRP, 128, GRP, EC * NQ], bf16,
                           kind="ExternalInput")
    wq8_d = nc.dram_tensor("wq8", [128, EC, EMB], f8, kind="ExternalInput")
    wk8_d = nc.dram_tensor("wk8", [128, EC, EMB], f8, kind="ExternalInput")
    wv_d = nc.dram_tensor("wv16", [128, EC, EMB], bf16, kind="ExternalInput")
    wout_d = nc.dram_tensor("wout16", [128, EC, ODIM], bf16,
                            kind="ExternalInput")
    pm_d = nc.dram_tensor("pmT", [NQ, BPC], f32, kind="ExternalInput")
    out_d = nc.dram_tensor("out", [BPC, NQ, ODIM], f32, kind="ExternalOutput")

    with tile.TileContext(nc) as tc:
        with (
            tc.tile_pool(name="const", bufs=1) as cpool,
            tc.tile_pool(name="gwork", bufs=2) as gwork,
            tc.tile_pool(name="work", bufs=3) as work,
            tc.tile_pool(name="nums", bufs=8) as nums,
            tc.tile_pool(name="ps", bufs=2, space="PSUM") as ps,
            tc.tile_pool(name="psl", bufs=2, space="PSUM") as psl_pool,
            tc.tile_pool(name="ps_att", bufs=2, space="PSUM") as ps_att,
        ):
            # ---- constants; batch 0's dependency chain (k weights,
            # e8[0], then q weights + group-0 q entities) is spread across
            # the four DMA queues and issued before anything else ----
            wq8_sb = cpool.tile([128, EC, EMB], f8)
            wk8_sb = cpool.tile([128, EC, EMB], f8)
            wv_sb = cpool.tile([128, EC, EMB], bf16)
            wout_sb = cpool.tile([128, EC, ODIM], bf16)
            pm_sb = cpool.tile([128, BPC], f32)
            # warmup/filler operand: a zeroed tile is enough (the psum is
            # never read); a 43ns memset beats make_identity's iota chain
            # that used to gate the PE's first instruction
            warm_sb = cpool.tile([128, 128], bf16)
            eps_sb = cpool.tile([128, 1], f32)
            ident = cpool.tile([128, 128], bf16)   # for the drain transpose

            def load_group0():
                """Group 0: per-batch contiguous DMAs, ordered for
                batch-0 latency (k first, then q, then v)."""
                e8_sb = gwork.tile([128, GRP, EC, NE], f8, name="e8_sb")
                e16_sb = gwork.tile([128, GRP, 4, EC, 128], bf16, name="e16_sb")
                eq8_sb = gwork.tile([128, EC, GRP, NQ], f8, name="eq8_sb")
                mask_sb = gwork.tile([128, GRP, EC * NQ], bf16, name="mask_sb")
                # each ring delivers transfers in prologue consumption
                # order (k, then q, then v): the DMA engines serve rings
                # round-robin, so a bulk transfer issued early steals
                # bandwidth from the batch-0 path; Scalar's ring starts
                # ~2us late behind its ACT_TABLE_LOAD preamble, so it
                # only carries batch 1-3 entities and the late constants
                nc.sync.dma_start(out=wk8_sb, in_=wk8_d.ap())
                nc.gpsimd.dma_start(out=e8_sb[:, 0], in_=kv8_d.ap()[0][:, 0])
                nc.sync.dma_start(out=wq8_sb, in_=wq8_d.ap())
                nc.gpsimd.dma_start(out=eq8_sb, in_=q8_d.ap()[0])
                nc.scalar.dma_start(out=wv_sb, in_=wv_d.ap())
                for cn in range(4):
                    nc.gpsimd.dma_start(out=e16_sb[:, 0, cn],
                                        in_=e16_d.ap()[0][:, 0, cn])
                for i in range(1, GRP):
                    nc.scalar.dma_start(out=e8_sb[:, i],
                                        in_=kv8_d.ap()[0][:, i])
                    nc.gpsimd.dma_start(out=e16_sb[:, i],
                                        in_=e16_d.ap()[0][:, i])
                nc.sync.dma_start(out=mask_sb, in_=m16_d.ap()[0])
                nc.sync.dma_start(out=wout_sb, in_=wout_d.ap())
                nc.scalar.dma_start(out=pm_sb, in_=pm_d.ap())
                return e8_sb, e16_sb, eq8_sb, mask_sb

            def load_group(g):
                """Prefetched groups: one contiguous DMA per tensor."""
                e8_sb = gwork.tile([128, GRP, EC, NE], f8, name="e8_sb")
                e16_sb = gwork.tile([128, GRP, 4, EC, 128], bf16, name="e16_sb")
                eq8_sb = gwork.tile([128, EC, GRP, NQ], f8, name="eq8_sb")
                mask_sb = gwork.tile([128, GRP, EC * NQ], bf16, name="mask_sb")
                nc.gpsimd.dma_start(out=e8_sb, in_=kv8_d.ap()[g])
                nc.scalar.dma_start(out=eq8_sb, in_=q8_d.ap()[g])
                nc.sync.dma_start(out=mask_sb, in_=m16_d.ap()[g])
                nc.gpsimd.dma_start(out=e16_sb[:, 0:2],
                                    in_=e16_d.ap()[g][:, 0:2])
                nc.gpsimd.dma_start(out=e16_sb[:, 2:4],
                                    in_=e16_d.ap()[g][:, 2:4])
                return e8_sb, e16_sb, eq8_sb, mask_sb

            def emit_logits_hc(i, hc, qT_sb, kT_sb):
                """Logits matmuls for one head pair into one 2-bank psl
                tile (row-group interleaved)."""
                pl = psl_pool.tile([128, 2, 4, NQ], f32, tag="psl",
                                   name="pl")
                for cn in range(4):
                    for r in range(2):
                        nc.tensor.matmul(
                            pl[:, r, cn, :],
                            lhsT=kT_sb[64 * r:64 * (r + 1), hc,
                                       128 * cn:128 * (cn + 1)],
                            rhs=qT_sb[64 * r:64 * (r + 1), hc, i, :],
                            start=True, stop=True)
                return pl

            def emit_exp(hc, pl):
                """one merged exp per head pair (Scalar)."""
                exp_sb = nums.tile([128, 8 * NQ], bf16, tag="exp",
                                   name="exp_sb")
                nc.scalar.activation(
                    exp_sb, pl.rearrange("p r c q -> p (r c q)"),
                    Exp, scale=EXP_SCALE)
                return exp_sb

            def emit_num(i, exp_sb, mask_sb):
                """one merged mask multiply per head pair (Vector); the
                mask broadcasts across the two row-group halves."""
                num_sb = nums.tile([128, 8 * NQ], bf16, tag="num",
                                   name="num_sb")
                nc.vector.tensor_mul(
                    num_sb.rearrange("p (r x) -> p r x", r=2),
                    exp_sb.rearrange("p (r x) -> p r x", r=2),
                    mask_sb[:, i, :].unsqueeze(1).broadcast_to(
                        (128, 2, EC * NQ)))
                return num_sb

            def qproj_chunks(grp_tiles):
                """fp8 DoubleRow qT projection for a whole group:
                qT[f, (i q)]; two PE chunks of 4 DR matmuls."""
                eq8_sb = grp_tiles[2]
                qT_sb = gwork.tile([128, 4, GRP, NQ], bf16, name="qT_sb")

                def chunk(cf0):
                    for cf in (cf0, cf0 + 1):
                        psum_q = ps.tile([128, GRP, NQ], f32, tag="big",
                                         name="psum_q")
                        for c in range(2):
                            nc.tensor.matmul(
                                psum_q,
                                lhsT=wq8_sb[:, 2 * c:2 * c + 2,
                                            128 * cf:128 * (cf + 1)],
                                rhs=eq8_sb[:, 2 * c:2 * c + 2, :, :],
                                start=(c == 0), stop=(c == 1), perf_mode=DR)
                        if cf < 2:
                            nc.vector.tensor_copy(qT_sb[:, cf, :, :], psum_q)
                        else:
                            nc.scalar.copy(qT_sb[:, cf, :, :], psum_q)

                return qT_sb, [lambda: chunk(0), lambda: chunk(2)]

            def kT_chunks(i, grp_tiles):
                """fp8 DoubleRow kT projection for batch slot i: kT[f, n];
                two PE chunks of 4 DR matmuls + 2 Vector copies each."""
                e8_sb = grp_tiles[0]
                kT_sb = work.tile([128, 4, NE], bf16, name="kT_sb")

                def chunk(cf):
                    psum_k = ps.tile([128, NE], f32, tag="big",
                                     name="psum_k")
                    for c in range(2):
                        nc.tensor.matmul(
                            psum_k,
                            lhsT=wk8_sb[:, 2 * c:2 * c + 2,
                                        128 * cf:128 * (cf + 1)],
                            rhs=e8_sb[:, i, 2 * c:2 * c + 2, :],
                            start=(c == 0), stop=(c == 1), perf_mode=DR)
                    nc.vector.tensor_copy(kT_sb[:, cf, :], psum_k)

                return kT_sb, [(lambda cf=cf: chunk(cf)) for cf in range(4)]

            def v_chunks(i, grp_tiles):
                """bf16 v projection (natural layout) + ones column; four PE
                chunks of 4 matmuls; copies cn0,1 on Vector, cn2,3 on Scalar
                (Scalar's land after the current batch's exp chain)."""
                e16_sb = grp_tiles[1]
                v_sb = work.tile([128, 4, H, HD + 1], bf16, name="v_sb")
                nc.gpsimd.memset(v_sb[:, :, :, HD], 1.0)

                def chunk(cn):
                    psum_v = ps.tile([128, EMB], f32, tag="big", name="psum_v")
                    for ce in range(EC):
                        nc.tensor.matmul(
                            psum_v,
                            lhsT=e16_sb[:, i, cn, ce, :],
                            rhs=wv_sb[:, ce, :],
                            start=(ce == 0), stop=(ce == EC - 1))
                    src = psum_v.rearrange("p (h d) -> p h d", h=H)
                    if cn != 2:
                        nc.vector.tensor_copy(v_sb[:, cn, :, 0:HD], src)
                    else:
                        nc.scalar.copy(v_sb[:, cn, :, 0:HD], src)

                return v_sb, [(lambda cn=cn: chunk(cn)) for cn in range(4)]

            def emit_attn(hc, num_sb, v_sb, att_tiles):
                for r in range(2):
                    h = 2 * hc + r
                    patt, j = att_tiles[h // 4], h % 4
                    for cn in range(4):
                        o = 512 * r + 128 * cn
                        nc.tensor.matmul(
                            patt[:, j, :],
                            lhsT=num_sb[:, o:o + 128],
                            rhs=v_sb[:, cn, h, :],
                            start=(cn == 0), stop=(cn == 3))

            def tail_v(b, att_tiles, mode="sync"):
                """denominators (S copies + V recip) + normalize (V) +
                transpose. GpSimd cannot read PSUM, so the psum reads all
                stay on Vector/Scalar; the transpose queue is selectable:
                steady state uses the Sync DMA xbar, batch 14 the GpSimd
                queue (Sync's sequencer picks the semaphore up late), and
                batch 15 the idle PE + one Vector copy."""
                dall_sb = work.tile([128, H], f32, name="dall_sb")
                nc.scalar.add(dall_sb[:, 0:4], att_tiles[0][:, :, HD], eps_sb)
                nc.scalar.add(dall_sb[:, 4:8], att_tiles[1][:, :, HD], eps_sb)
                recip_sb = work.tile([128, H], f32, name="recip_sb")
                nc.vector.reciprocal(recip_sb, dall_sb)

                attn_sb = work.tile([128, EMB], bf16, name="attn_sb")
                for t in range(2):
                    nc.vector.tensor_mul(
                        attn_sb[:, 256 * t:256 * (t + 1)]
                            .rearrange("p (h d) -> p h d", h=4),
                        att_tiles[t][:, :, 0:HD],
                        recip_sb[:, 4 * t:4 * t + 4].unsqueeze(2)
                            .broadcast_to((128, 4, HD)))
                if mode == "none":
                    return attn_sb          # caller transposes via the PE
                attnT_sb = work.tile([128, 4, 128], bf16, name="attnT_sb")
                nc.sync.dma_start_transpose(attnT_sb, attn_sb)
                return attnT_sb

            def pe_transpose(attn_sb):
                """attn [q, emb] -> attnT [emb, 4, q] on the (drain-idle)
                PE + one Vector copy; avoids the Sync DMA xbar whose
                sequencer services the tail transposes several us late."""
                psT = ps.tile([128, 4, 128], bf16, tag="big", name="psT")
                for ct in range(4):
                    nc.tensor.transpose(
                        psT[:, ct, :], attn_sb[:, 128 * ct:128 * (ct + 1)],
                        ident)
                attnT_sb = work.tile([128, 4, 128], bf16, name="attnT_sb")
                nc.vector.tensor_copy(attnT_sb, psT)
                return attnT_sb

            def out_proj(b, attnT_sb):
                psum_o = ps.tile([128, ODIM], f32, tag="big", name="psum_o")
                for ct in range(4):
                    nc.tensor.matmul(
                        psum_o,
                        lhsT=attnT_sb[:, ct, :],
                        rhs=wout_sb[:, ct, :],
                        start=(ct == 0), stop=(ct == 3))
                out_sb = work.tile([128, ODIM], f32, name="out_sb")
                if b == BPC - 1:
                    # final batch: halve the fully-exposed tail chain and
                    # spread the last DMA over four queues
                    nc.scalar.mul(out_sb[:, 0:256], psum_o[:, 0:256],
                                  pm_sb[:, b:b + 1])
                    nc.vector.tensor_scalar_mul(
                        out_sb[:, 256:], psum_o[:, 256:], pm_sb[:, b:b + 1])
                    nc.gpsimd.dma_start(out=out_d.ap()[b, :, 0:192],
                                        in_=out_sb[:, 0:192])
                    nc.scalar.dma_start(out=out_d.ap()[b, :, 192:320],
                                        in_=out_sb[:, 192:320])
                    nc.sync.dma_start(out=out_d.ap()[b, :, 320:512],
                                      in_=out_sb[:, 320:512])
                else:
                    nc.scalar.mul(out_sb, psum_o, pm_sb[:, b:b + 1])
                    nc.gpsimd.dma_start(out=out_d.ap()[b], in_=out_sb)

            # ---- software pipeline (one-period attention skew) ----
            # Period b emits: logits(b) + exp(b), the projection chunks for
            # b+1, the attention matmuls of b-1 (whose num tiles were
            # finished last period - so they never stall the PE), num(b),
            # tail_v(b-1), out_proj(b-2). A drain period finishes b=15.
            ngrp = BPC // GRP
            grp_tiles = [None] * ngrp
            qT = [None] * ngrp
            # the warmup operand memset goes first (43ns), then the
            # group-0 DMA issues, so the critical wk8/e8[0] transfers
            # start the moment the queues come up
            nc.gpsimd.memset(warm_sb, 0.0)
            grp_tiles[0] = load_group0()
            nc.gpsimd.memset(eps_sb, 1e-25)

            # warm-up matmuls: keep the PE HAM at full clock while the
            # first weight and entity DMAs are in flight
            psum_w = ps.tile([128, 128], f32, tag="big", name="psum_w")
            for _ in range(68):
                nc.tensor.matmul(psum_w, lhsT=warm_sb, rhs=warm_sb,
                                 start=True, stop=True)

            # prologue: projections for batch 0, k first (its operands
            # arrive first; Scalar's ACT_TABLE_LOAD delays the q loads)
            kT_cur, kc = kT_chunks(0, grp_tiles[0])
            qT[0], qc = qproj_chunks(grp_tiles[0])
            v_prev = None
            v_cur, vc = v_chunks(0, grp_tiles[0])
            for c in kc + qc + vc:
                c()
            # identity for the drain's PE transpose; the iota chain runs
            # while GpSimd is otherwise idle
            make_identity(nc, ident)

            def filler(n):
                fill = ps.tile([128, 128], f32, tag="big", name="fill")
                for _ in range(n):
                    nc.tensor.matmul(fill, lhsT=warm_sb, rhs=warm_sb,
                                     start=True, stop=True)

            nums_prev = None
            T1 = None             # attnT of b-2 at period start
            T14 = None            # attnT of batch 14 (tail run in period 14)
            for b in range(BPC - 1):
                g, i = divmod(b, GRP)
                mask_sb = grp_tiles[g][3]

                # projection chunks for b+1
                chunks = []
                if b + 1 < BPC:
                    if i == 0 and g + 1 < ngrp:
                        grp_tiles[g + 1] = load_group(g + 1)
                    g1, i1 = divmod(b + 1, GRP)
                    kT_next, kc = kT_chunks(i1, grp_tiles[g1])
                    v_next, vc = v_chunks(i1, grp_tiles[g1])
                    if i1 == 0:
                        qT[g1], qc = qproj_chunks(grp_tiles[g1])
                        chunks = [qc[0], kc[0], qc[1]] + kc[1:] + vc
                    else:
                        chunks = kc + vc
                else:
                    kT_next = v_next = None
                nci = 0

                def next_chunk():
                    nonlocal nci
                    if nci < len(chunks):
                        chunks[nci]()
                        nci += 1

                pls, exps, nums_cur = [], [], []
                for hc in range(4):
                    pls.append(emit_logits_hc(i, hc, qT[g], kT_cur))
                    exps.append(emit_exp(hc, pls[hc]))
                    next_chunk()
                if b > 0:
                    att_tiles = [
                        ps_att.tile([128, 4, HD + 1], f32, tag="att",
                                    name="pa0"),
                        ps_att.tile([128, 4, HD + 1], f32, tag="att",
                                    name="pa1"),
                    ]
                for hc in range(4):
                    if b > 0:
                        emit_attn(hc, nums_prev[hc], v_prev, att_tiles)
                    next_chunk()
                while nci < len(chunks):
                    next_chunk()
                # num-muls AFTER all psum->sbuf copies: the copies gate
                # psum-bank recycling (the PE's next start=True stalls on
                # the copy two allocations back), while the num-muls'
                # consumer (attn of this batch) only runs early next
                # period, so they can wait at the back of the V queue
                for hc in range(4):
                    nums_cur.append(emit_num(i, exps[hc], mask_sb))

                if b > 0:
                    T0 = tail_v(b - 1, att_tiles)
                    if T1 is not None:
                        out_proj(b - 2, T1)
                    T1 = T0
                if b == BPC - 2:
                    # shift attn(14) + tail(14) into period 14: num(14) is
                    # complete before the PE works through the ~9us of
                    # logits/attn(13)/projection work that precede it, so
                    # attnT(14) (GpSimd-queue transpose; the Sync
                    # sequencer serviced it several us late) lands early
                    # in period 15 and the drain only carries batch 15
                    att14_tiles = [
                        ps_att.tile([128, 4, HD + 1], f32, tag="att",
                                    name="pa0"),
                        ps_att.tile([128, 4, HD + 1], f32, tag="att",
                                    name="pa1"),
                    ]
                    for hc in range(4):
                        emit_attn(hc, nums_cur[hc], v_cur, att14_tiles)
                    A14 = tail_v(b, att14_tiles, mode="none")
                nums_prev = nums_cur
                kT_cur = kT_next
                v_prev, v_cur = v_cur, v_next

            # ---- period 15 + drain, explicit ----
            # attn(14)/tail(14) already ran in period 14, so this period
            # only carries logits/exp/num(15), the two deferred output
            # projections, and batch 15's own attention + tail; fillers
            # pace the PE between dependency-gated steps so the HAM
            # clock gate stays open
            mask_sb = grp_tiles[ngrp - 1][3]
            pls15, exps15, nums15 = [], [], []
            for hc in range(4):
                pls15.append(
                    emit_logits_hc(GRP - 1, hc, qT[ngrp - 1], kT_cur))
                exps15.append(emit_exp(hc, pls15[hc]))
                if hc == 1:
                    filler(10)                 # attnT(13) lands meanwhile
                    out_proj(BPC - 3, T1)
            for hc in range(4):
                nums15.append(emit_num(GRP - 1, exps15[hc], mask_sb))
            att_tiles = [
                ps_att.tile([128, 4, HD + 1], f32, tag="att", name="pa0"),
                ps_att.tile([128, 4, HD + 1], f32, tag="att", name="pa1"),
            ]
            filler(14)
            emit_attn(0, nums15[0], v_cur, att_tiles)
            emit_attn(1, nums15[1], v_cur, att_tiles)
            T14 = pe_transpose(A14)
            emit_attn(2, nums15[2], v_cur, att_tiles)
            filler(10)
            emit_attn(3, nums15[3], v_cur, att_tiles)
            filler(8)
            A15 = tail_v(BPC - 1, att_tiles, mode="none")
            T0 = pe_transpose(A15)
            out_proj(BPC - 2, T14)
            filler(4)
            out_proj(BPC - 1, T0)

    nc.compile()
    return nc


def _prep_inputs(entities, pre_mask, diff_mask, post_mask, W_in, W_out):
    entities = np.asarray(entities, dtype=np.float32)
    pre_mask = np.asarray(pre_mask, dtype=bool)
    diff_mask = np.asarray(diff_mask, dtype=np.float32)
    post_mask = np.asarray(post_mask, dtype=bool)
    W_in = np.asarray(W_in, dtype=np.float32)
    W_out = np.asarray(W_out, dtype=np.float32)

    entsT = np.ascontiguousarray(entities.transpose(0, 2, 1))  # [BS, DIN, NE]
    ents8T = (entsT * SE).astype(F8)
    ents16T = entsT.astype(BF16)
    m = (~pre_mask).astype(np.float32) * (diff_mask + 1e-8)
    maskT = np.ascontiguousarray(m.transpose(0, 2, 1)).astype(BF16)
    w_inT = np.ascontiguousarray(W_in.T)           # [DIN, 3E]
    w_qk8T = (w_inT[:, :2 * EMB] * SW).astype(F8)
    w_v16T = np.ascontiguousarray(w_inT[:, 2 * EMB:]).astype(BF16)
    w_outT = np.ascontiguousarray(W_out.T).astype(BF16)
    pmT = np.ascontiguousarray((~post_mask).T.astype(np.float32))

    def pack_bcen(x):
        # [BPC, DIN, X] -> [NGRP, 128, GRP, EC, X]; (g,p,i,c,x) =
        # src[g*GRP+i, c*128+p, x]
        X = x.shape[2]
        return np.ascontiguousarray(
            x.reshape(NGRP, GRP, EC, 128, X).transpose(0, 3, 1, 2, 4))

    def pack_w(w):
        # [DIN, F] -> [128, EC, F]
        return np.ascontiguousarray(
            w.reshape(EC, 128, -1).transpose(1, 0, 2))

    wq8 = pack_w(w_qk8T[:, :EMB])
    wk8 = pack_w(w_qk8T[:, EMB:])
    wv16 = pack_w(w_v16T)
    wout16 = pack_w(w_outT)

    in_maps = []
    for c in range(NCORES):
        sl = slice(c * BPC, (c + 1) * BPC)
        e8c = ents8T[sl]
        q8 = np.ascontiguousarray(
            e8c[:, :, :NQ].reshape(NGRP, GRP, EC, 128, NQ)
               .transpose(0, 3, 2, 1, 4))          # [NGRP,128,EC,GRP,NQ]
        m16 = pack_bcen(maskT[sl]).reshape(NGRP, 128, GRP, EC * NQ)
        in_maps.append({
            "kv8": pack_bcen(e8c),
            "e16": np.ascontiguousarray(
                ents16T[sl].reshape(NGRP, GRP, EC, 128, 4, 128)
                           .transpose(0, 3, 1, 4, 2, 5)),
            "q8": q8,
            "m16": np.ascontiguousarray(m16),
            "wq8": wq8,
            "wk8": wk8,
            "wv16": wv16,
            "wout16": wout16,
            "pmT": np.ascontiguousarray(pmT[:, sl]),
        })
    return in_maps


def _run(in_maps, trace=False):
    from concourse.bass_utils import run_bass_kernel_spmd
    nc = _build_nc()
    last_exc = None
    for attempt in range(3):
        try:
            return run_bass_kernel_spmd(
                nc, in_maps, core_ids=list(range(NCORES)), trace=trace)
        except Exception as e:  # transient NRT_EXEC_UNIT faults on fresh NEFFs
            last_exc = e
            import time
            time.sleep(2.0 * (attempt + 1))
    raise last_exc


def kernel_traced(entities, pre_mask, diff_mask, post_mask, W_in, W_out, b_out,
                  trace=False):
    """Returns (output, BassKernelResults)."""
    b_out = np.asarray(b_out, dtype=np.float32)
    post_mask_np = np.asarray(post_mask, dtype=bool)
    in_maps = _prep_inputs(entities, pre_mask, diff_mask, post_mask, W_in, W_out)
    res = _run(in_maps, trace=trace)
    out = np.concatenate([r["out"] for r in res.results], axis=0)
    # faithfulness: reference adds b_out before the post-mask zeroing
    out = out + np.where(post_mask_np[:, :, None], 0.0, b_out[None, None, :])
    return out.astype(np.float32), res


def kernel(entities, pre_mask, diff_mask, post_mask, W_in, W_out, b_out):
    out, _ = kernel_traced(entities, pre_mask, diff_mask, post_mask,
                           W_in, W_out, b_out)
    return out
